# revision 27
# baseline (speedup 1.0000x reference)
"""Trainium2 Bass kernel for nn_BiaffineNER (BiDAF attention + FFW + biaffine scorer).

Contract: kernel(**inputs) takes the FULL unsharded inputs (numpy) and returns
the FULL [16, 512, 512, 3] float32 output. Internally shards data-parallel over
the batch axis across 8 NeuronCores (2 batch elements per core), runs one SPMD
Bass/Tile program on all cores, and concatenates the per-core outputs.

Math per batch element b (LC=512, LQ=64, H=256, D=4H=1024, DFF=512, C=3):
  sim  = (ctx@w1)[:,None] + (ques@w2)[None,:] + (ctx*w3)@ques.T      [LC,LQ]
  a    = softmax_j(sim); c2q = a @ ques                              [LC,H]
  bwt  = softmax_i(max_j sim); q2c = bwt @ ctx                       [H]
  x    = [ctx, c2q, ctx*c2q, ctx*q2c]                                [LC,D]
  start= relu(x@W1s+b1s)@W2s+b2s ; end likewise                      [LC,D]
  out[x,y,c] = [start,1][x] . Wb[:,c,:] . [end,1][y]                 [LC,LC,C]

Design notes:
- Activations kept transposed on-chip ([feature-part, token-free]) so the
  contraction dim always sits on SBUF partitions; ctx^T/ques^T come in
  host-pretransposed.
- All matmul operands are bfloat16: the PE runs bf16 at 1 cycle/row (full
  rate; fp32r pays a serialized 4-byte LDWEIGHTS ~176ns/matmul, and IEEE fp16
  measures 2 cycles/row on real HW).  End-to-end rel err ~6e-3 (tolerance
  2e-2); accumulation stays fp32 in PSUM.
- All weights (FFW + the 12.6MB Wb) are loaded once in bf16 and stay resident
  in SBUF for both batch elements: HBM read drops ~44MB -> ~12MB per core.
- dma_start issue cost is ~0.6-0.75us of engine time each, so DMAs are merged
  aggressively (host arrays pre-packed p-major so merged DMAs stay contiguous):
  2 constant packs, 1 DMA per FFW weight matrix, 1 DMA for all of Wb, 4 DMAs
  per batch of inputs, one output DMA per (batch, label) plane group.
- DMA queue plan: each hw queue (scalar/Activation, sync/SP) carries one
  batch's critical inputs FIRST; all 10.3MB of bulk weights then ride the
  sync queue alone, ordered by first use (the Activation queue only sustains
  ~90GB/s while sync's is active; sync alone does ~290GB/s).  Output planes
  also leave via sync.  gpsimd's software queue (~43GB/s) carries nothing.
- The output leaves as bf16 [C, LC, LC] planes (host transposes/upcasts for
  free), so the kernel tail is one 512KB DMA, not a whole batch element.
- Softmaxes skip max-subtraction (|sim| < ~8 for this data distribution), which
  turns the partition-axis softmax over i into tiny matmul reductions.
- The two batch elements' attention front-ends are instruction-interleaved
  (independent dependency chains hide each other's latency), then
  F0 F1 B0 B1, with a ~3us dependency-free warm-up matmul burst up front so
  the PE HAM clock gate opens before the first dense phase.
"""

import sys

if "/opt/trn_rl_repo" not in sys.path and "/root/.axon_site/_ro/trn_rl_repo" not in sys.path:
    sys.path.insert(0, "/opt/trn_rl_repo")

import numpy as np

import concourse.bass as bass
import concourse.tile as tile
from concourse import bacc, mybir

F32 = mybir.dt.float32
F16 = mybir.dt.bfloat16
AF = mybir.ActivationFunctionType
ALU = mybir.AluOpType
AX = mybir.AxisListType

N_CORES = 8
B, LC, LQ, H = 16, 512, 64, 256
NB = B // N_CORES          # batch elements per core
D, DFF, C = 4 * H, 512, 3
NIC = LC // 128            # 4  i/x chunks
NHC = H // 128             # 2  h chunks
NDC = D // 128             # 8  d chunks
NFC = DFF // 128           # 4  f chunks
NJC = 8                    # j chunks (first 1024 of 1025)

# f32 constant pack column layout [128, 54]
COL_W3 = 0                 # 2 cols (w3 chunks, per-partition scalars)
COL_B1S, COL_B2S = 2, 6    # 4 + 8
COL_B1E, COL_B2E = 14, 18  # 4 + 8
COL_VC = 26                # 24 cols: vcols[c*NJC + jc]
NF32 = 50

# bf16 constant pack column layout [128, 687]
OFF_ONES = 0               # 512 cols of 1.0 (ones_row row 0; ones2 any 2 cols)
OFF_COLW = 512             # 8 cols: [w1_0, 0, w1_1, 0, 0, w2_0, 0, w2_1]
OFF_IDENT = 520            # 128 cols identity
OFF_UPACK = 648            # 32 cols: upack[dc*4 + c] = Wb[dc*128+p, c, D]
OFF_WROW4 = 680            # 4 cols: Wb[D, c, D] broadcast down partitions
OFF_Z = 684                # 3 cols [1, 0, 1]: [z:z+2]=[1,0], [z+1:z+3]=[0,1]
NBF16 = 687


def _build_program():
    nc = bacc.Bacc("TRN2", target_bir_lowering=False, debug=False,
                   num_devices=N_CORES)

    ctx_d = nc.dram_tensor("ctx", [NB, 128, NIC, H], F16, kind="ExternalInput").ap()
    ques_d = nc.dram_tensor("ques", [NB, LQ, H], F16, kind="ExternalInput").ap()
    ctxT_d = nc.dram_tensor("ctxT", [NB, 128, NHC, LC], F16, kind="ExternalInput").ap()
    quesT_d = nc.dram_tensor("quesT", [NB, 128, NHC, LQ], F16, kind="ExternalInput").ap()
    f32p_d = nc.dram_tensor("f32pack", [128, NF32], F32, kind="ExternalInput").ap()
    bf16p_d = nc.dram_tensor("bf16pack", [128, NBF16], F16, kind="ExternalInput").ap()
    wb_d = nc.dram_tensor("wb", [128, NDC, C, D + 1], F16, kind="ExternalInput").ap()
    w1s_d = nc.dram_tensor("W1s", [128, NDC, DFF], F16, kind="ExternalInput").ap()
    w2s_d = nc.dram_tensor("W2s", [128, NFC, D], F16, kind="ExternalInput").ap()
    w1e_d = nc.dram_tensor("W1e", [128, NDC, DFF], F16, kind="ExternalInput").ap()
    w2e_d = nc.dram_tensor("W2e", [128, NFC, D], F16, kind="ExternalInput").ap()
    out_d = nc.dram_tensor("out", [NB, C, 128, NIC, LC], F16, kind="ExternalOutput").ap()

    with tile.TileContext(nc) as tc:
        _trace_kernel(nc, tc, ctx_d, ques_d, ctxT_d, quesT_d, f32p_d, bf16p_d,
                      wb_d, (w1s_d, w2s_d), (w1e_d, w2e_d), out_d)
    nc.compile()
    return nc


def _trace_kernel(nc, tc, ctx_d, ques_d, ctxT_d, quesT_d, f32p_d, bf16p_d,
                  wb_d, ws_d, we_d, out_d):
    import contextlib
    est = contextlib.ExitStack()
    with est:
        const = est.enter_context(tc.tile_pool(name="const", bufs=1))
        attn = est.enter_context(tc.tile_pool(name="attn", bufs=1))
        wres = est.enter_context(tc.tile_pool(name="wres", bufs=1))
        tring = est.enter_context(tc.tile_pool(name="tring", bufs=9))
        acts = est.enter_context(tc.tile_pool(name="acts", bufs=1))
        oplane = est.enter_context(tc.tile_pool(name="oplane", bufs=3))
        cols = est.enter_context(tc.tile_pool(name="cols", bufs=2))
        pmm = est.enter_context(tc.tile_pool(name="pmm", bufs=3, space="PSUM"))
        pffw = est.enter_context(tc.tile_pool(name="pffw", bufs=3, space="PSUM"))
        ptiny = est.enter_context(tc.tile_pool(name="ptiny", bufs=2, space="PSUM"))
        pat = ptiny

        def mm(out, lhsT, rhs, start, stop):
            nc.tensor.matmul(out, lhsT, rhs, start=start, stop=stop)

        # HAM warm-up: ~3us of dependency-free PE activity (plain fp32, fed by
        # a memset tile) so the clock gate opens before the first dense phase.
        ones2_f = const.tile([128, 2], F32, tag="ones2_f")
        nc.vector.memset(ones2_f[:], 1.0)
        p_warm = pmm.tile([128, 512], F32, tag="pmm")
        for wi in range(60):
            nc.tensor.matmul(p_warm[0:2, 0:2], ones2_f[:], ones2_f[:],
                             start=(wi == 0), stop=(wi == 59))

        # ---- DMA plan: each hw queue carries one batch's critical inputs
        # FIRST (plus one const pack), then its share of the bulk weights.
        # Criticals on both queues drain in parallel at full HBM rate before
        # any bulk weight competes for bandwidth.
        f32p = const.tile([128, NF32], F32, tag="f32p")
        nc.scalar.dma_start(out=f32p[:], in_=f32p_d[:])
        bf16p = const.tile([128, NBF16], F16, tag="bf16p")
        nc.sync.dma_start(out=bf16p[:], in_=bf16p_d[:])

        colw = bf16p[:, OFF_COLW:OFF_COLW + 8]
        ident = bf16p[:, OFF_IDENT:OFF_IDENT + 128]
        ones_row = bf16p[0:1, OFF_ONES:OFF_ONES + 512]
        ones2 = bf16p[:, OFF_ONES:OFF_ONES + 2]
        wrow4 = bf16p[0:1, OFF_WROW4:OFF_WROW4 + 4]

        quesT_sb, ctxT_sb, ques_sb, ctx_sb = {}, {}, {}, {}
        for b, eng in ((0, nc.scalar), (1, nc.sync)):
            t_ = attn.tile([128, NHC, LQ], F16, tag=f"quesT_{b}", name=f"quesT_{b}")
            eng.dma_start(out=t_[:], in_=quesT_d[b])
            quesT_sb[b] = [t_[:, hc, :] for hc in range(NHC)]
            t_ = attn.tile([128, NHC, LC], F16, tag=f"ctxT_{b}", name=f"ctxT_{b}")
            eng.dma_start(out=t_[:], in_=ctxT_d[b])
            ctxT_sb[b] = [t_[:, hc, :] for hc in range(NHC)]
            q_ = attn.tile([LQ, H], F16, tag=f"ques_{b}", name=f"ques_{b}")
            eng.dma_start(out=q_[:], in_=ques_d[b, :, :])
            ques_sb[b] = q_
            t_ = attn.tile([128, NIC, H], F16, tag=f"ctx_{b}", name=f"ctx_{b}")
            eng.dma_start(out=t_[:], in_=ctx_d[b])
            ctx_sb[b] = [t_[:, ic, :] for ic in range(NIC)]

        # bulk weights, after the criticals, ALL on the sync queue ordered by
        # first use (w1s ~20us ... Wb ~80us).  The scalar (Activation) hw
        # queue only gets ~90GB/s when sync's queue is active, so bulk on it
        # arrives late; sync alone moves 10.3MB well before each deadline.
        wtiles = {}
        for lname, (w1_d, w2_d) in (("s", ws_d), ("e", we_d)):
            w1t = wres.tile([128, NDC, DFF], F16, tag=f"w1{lname}", name=f"w1{lname}")
            nc.sync.dma_start(out=w1t[:], in_=w1_d[:])
            w2t = wres.tile([128, NFC, D], F16, tag=f"w2{lname}", name=f"w2{lname}")
            nc.sync.dma_start(out=w2t[:], in_=w2_d[:])
            wtiles[lname] = (w1t, w2t)
        wbt_tile = wres.tile([128, NDC, C, D + 1], F16, tag="wb")
        nc.sync.dma_start(out=wbt_tile[:], in_=wb_d[:])

        def attention_pair():
            """Both batches' attention, instruction-interleaved stage by stage.
            Returns {b: xT chunk list} (8 tiles [128, LC] bf16 each)."""
            BS = (0, 1)
            quesT, ctxT = quesT_sb, ctxT_sb

            # (ctx*w3)^T
            ctxw3T = {b: [] for b in BS}
            for b in BS:
                for hc in range(NHC):
                    t_ = attn.tile([128, LC], F16, tag=f"ctxw3T{hc}_{b}")
                    nc.gpsimd.tensor_scalar_mul(
                        t_[:], ctxT[b][hc],
                        f32p[:, COL_W3 + hc:COL_W3 + hc + 1])
                    ctxw3T[b].append(t_)

            # Stacked rank-2 tiles, built wholly in PSUM via zero-padded
            # weight columns + a rank-1 ones term: q2ones = [ones; ques@w2],
            # c1ones = [ctx@w1; ones].  Both broadcast terms of sim are then
            # ONE matmul: c1ones[:,isl].T @ q2ones = c1[i]*1 + 1*q2[j].
            o10 = bf16p[0:1, OFF_Z:OFF_Z + 2]
            o01 = bf16p[0:1, OFF_Z + 1:OFF_Z + 3]
            q2ones, c1ones = {}, {}
            for b in BS:
                p_q2r = pat.tile([2, LQ], F32, tag="pt", name=f"pq2r_{b}")
                for hc in range(NHC):
                    mm(p_q2r[:], colw[:, 4 + 2 * hc:6 + 2 * hc], quesT[b][hc],
                       start=(hc == 0), stop=False)
                mm(p_q2r[:], o10, ones_row[:, 0:LQ], start=False, stop=True)
                q2ones[b] = cols.tile([2, LQ], F16, tag="q2row", name=f"q2row_{b}")
                nc.vector.tensor_copy(q2ones[b][:], p_q2r[:])
            for b in BS:
                p_c1r = pffw.tile([2, LC], F32, tag="pf", name=f"pc1r_{b}")
                for hc in range(NHC):
                    mm(p_c1r[:], colw[:, 2 * hc:2 * hc + 2], ctxT[b][hc],
                       start=(hc == 0), stop=False)
                mm(p_c1r[:], o01, ones_row[:, 0:LC], start=False, stop=True)
                c1ones[b] = cols.tile([2, LC], F16, tag="c1row", name=f"c1row_{b}")
                nc.vector.tensor_copy(c1ones[b][:], p_c1r[:])

            ucols = {b: attn.tile([128, NIC + 2], F16, tag=f"ucols_{b}",
                                  name=f"ucols_{b}") for b in BS}
            a_n = {b: [] for b in BS}
            for ic in range(NIC):
                icsl = slice(ic * 128, (ic + 1) * 128)
                for b in BS:
                    p_sim = pmm.tile([128, LQ], F32, tag="pmm", name=f"psim_{b}{ic}")
                    for hc in range(NHC):
                        mm(p_sim[:], ctxw3T[b][hc][:, icsl], quesT[b][hc],
                           start=(hc == 0), stop=False)
                    mm(p_sim[:], c1ones[b][:, icsl], q2ones[b][:],
                       start=False, stop=True)

                    a_un = attn.tile([128, LQ], F32, tag=f"aun{ic}_{b}")
                    nc.scalar.activation(a_un[:], p_sim[:], AF.Exp)
                    ssum = cols.tile([128, 1], F32, tag="ssum", name=f"ssum_{b}{ic}")
                    nc.vector.reduce_sum(out=ssum[:], in_=a_un[:], axis=AX.X)
                    srec = cols.tile([128, 1], F32, tag="srec", name=f"srec_{b}{ic}")
                    nc.vector.reciprocal(srec[:], ssum[:])
                    nc.vector.reduce_max(out=ucols[b][:, ic:ic + 1], in_=a_un[:], axis=AX.X)
                    t_ = attn.tile([128, LQ], F16, tag=f"an{ic}_{b}")
                    nc.vector.tensor_scalar_mul(t_[:], a_un[:], srec[:])
                    a_n[b].append(t_)

            # a^T [j-part, i-free]
            aT = {b: attn.tile([LQ, LC], F16, tag=f"aT_{b}", name=f"aT_{b}")
                  for b in BS}
            for b in BS:
                for ic in range(NIC):
                    p = pffw.tile([LQ, 128], F16, tag="pf", name=f"paT_{b}{ic}")
                    nc.tensor.transpose(p[:], a_n[b][ic][:], ident)
                    nc.scalar.activation(aT[b][:, ic * 128:(ic + 1) * 128], p[:], AF.Copy)

            # softmax-over-i weights: denominator + broadcast of 1/den
            invb = {}
            for b in BS:
                ucol1 = cols.tile([128, 1], F16, tag="ucol1", name=f"ucol1_{b}")
                with nc.allow_low_precision(reason="4-term bf16 softmax-denominator partial sum"):
                    nc.vector.reduce_sum(out=ucol1[:], in_=ucols[b][:, 0:NIC], axis=AX.X)
                p_den = pat.tile([1, 2], F32, tag="pt", name=f"pden_{b}")
                mm(p_den[:], ucol1[:], ones2, start=True, stop=True)
                inv2f = cols.tile([1, 2], F32, tag="inv2f", name=f"inv2f_{b}")
                nc.vector.reciprocal(inv2f[:], p_den[:])
                inv2 = cols.tile([1, 2], F16, tag="inv2", name=f"inv2_{b}")
                nc.scalar.activation(inv2[:], inv2f[:], AF.Copy)
                p_bc = pat.tile([128, 2], F32, tag="pt", name=f"pbc_{b}")
                mm(p_bc[:], ones_row[:, 0:128], inv2[:], start=True, stop=True)
                invb[b] = cols.tile([128, 1], F32, tag="invb", name=f"invb_{b}")
                nc.scalar.activation(invb[b][:], p_bc[:, 0:1], AF.Copy)

            q2cc = {b: [] for b in BS}
            for b in BS:
                for hs in range(NHC):
                    p_q2c = pat.tile([128, 2], F32, tag="pt", name=f"pq2c_{b}{hs}")
                    for ic in range(NIC):
                        mm(p_q2c[:], ctx_sb[b][ic][:, hs * 128:(hs + 1) * 128],
                           ucols[b][:, ic:ic + 2], start=(ic == 0), stop=(ic == NIC - 1))
                    t_ = cols.tile([128, 1], F32, tag=f"q2cc{hs}", name=f"q2cc_{b}{hs}")
                    nc.vector.tensor_mul(t_[:], p_q2c[:, 0:1], invb[b][:])
                    q2cc[b].append(t_)

            # x^T chunks: 0-1 ctx^T, 2-3 c2q^T, 4-5 (ctx*c2q)^T, 6-7 (ctx*q2c)^T
            xT = {}
            for b in BS:
                xT[b] = [ctxT[b][0], ctxT[b][1]]
                for hs in range(NHC):
                    p_c2q = pffw.tile([128, LC], F32, tag="pf", name=f"pc2q_{b}{hs}")
                    mm(p_c2q[:], ques_sb[b][:, hs * 128:(hs + 1) * 128], aT[b][:],
                       start=True, stop=True)
                    t_ = acts.tile([128, LC], F16, tag=f"xT{2 + hs}_{b}")
                    nc.scalar.activation(t_[:], p_c2q[:], AF.Copy)
                    xT[b].append(t_)
                for hc in range(NHC):
                    t_ = acts.tile([128, LC], F16, tag=f"xT{4 + hc}_{b}")
                    nc.gpsimd.tensor_mul(t_[:], ctxT[b][hc], xT[b][2 + hc][:])
                    xT[b].append(t_)
                for hc in range(NHC):
                    t_ = acts.tile([128, LC], F16, tag=f"xT{6 + hc}_{b}")
                    nc.gpsimd.tensor_scalar_mul(t_[:], ctxT[b][hc], q2cc[b][hc][:])
                    xT[b].append(t_)
            return xT

        def ffw(b, xT):
            sT, eT = [], []
            for lname, colb1, colb2, dst in (
                ("s", COL_B1S, COL_B2S, sT),
                ("e", COL_B1E, COL_B2E, eT),
            ):
                w1t, w2t = wtiles[lname]
                h1 = []
                dc_order = [0, 1, 6, 7, 2, 3, 4, 5]
                for fc in range(NFC):
                    p = pffw.tile([128, LC], F32, tag="pf", name=f"ph1{lname}_{b}{fc}")
                    for k, dc in enumerate(dc_order):
                        mm(p[:], w1t[:, dc, fc * 128:(fc + 1) * 128], xT[dc],
                           start=(k == 0), stop=(k == NDC - 1))
                    t_ = acts.tile([128, LC], F16, tag=f"h1{fc}",
                                   name=f"h1{lname}{fc}_{b}")
                    nc.vector.tensor_scalar(
                        out=t_[:], in0=p[:],
                        scalar1=f32p[:, colb1 + fc:colb1 + fc + 1],
                        scalar2=0.0, op0=ALU.add, op1=ALU.max)
                    h1.append(t_)
                for dc in range(NDC):
                    p = pffw.tile([128, LC], F32, tag="pf", name=f"po{lname}_{b}{dc}")
                    for fc in range(NFC):
                        mm(p[:], w2t[:, fc, dc * 128:(dc + 1) * 128], h1[fc][:],
                           start=(fc == 0), stop=(fc == NFC - 1))
                    t_ = acts.tile([128, LC], F16, tag=f"{lname}T{dc}", bufs=2,
                                   name=f"{lname}T{dc}_{b}")
                    nc.scalar.activation(
                        t_[:], p[:], AF.Identity,
                        bias=f32p[:, colb2 + dc:colb2 + dc + 1],
                        scale=1.0)
                    dst.append(t_)
            return sT, eT

        def biaffine(b, sT, eT):
            # t1 rows for all three labels in one group:
            # t1[c, x] = sum_i start^T[i, x] * Wb[i, c, D]  + Wb[D, c, D]
            p_t14 = pffw.tile([4, LC], F32, tag="pf", name=f"pt14_{b}")
            for ic in range(NDC):
                mm(p_t14[:], bf16p[:, OFF_UPACK + ic * 4:OFF_UPACK + ic * 4 + 4],
                   sT[ic][:], start=(ic == 0), stop=False)
            mm(p_t14[:], wrow4, ones_row, start=False, stop=True)
            t14 = cols.tile([4, LC], F16, tag="t14", name=f"t14_{b}")
            nc.scalar.activation(t14[:], p_t14[:], AF.Copy)
            t1cols = []
            for xc in range(NIC):
                p = ptiny.tile([128, 4], F16, tag="pt", name=f"pt1c_{b}{xc}")
                nc.tensor.transpose(p[:], t14[:, xc * 128:(xc + 1) * 128],
                                    ident[0:4, 0:4])
                tsb = cols.tile([128, 4], F32, tag=f"t1c{xc}", name=f"t1c{xc}_{b}")
                nc.vector.tensor_copy(tsb[:], p[:])
                t1cols.append(tsb)

            for c in range(C):
                # t_c^T[j, x] = sum_i Wb[i,c,j] * start^T[i, x]  (+ v_c[j])
                tt = []
                for jc in range(NJC):
                    p = pmm.tile([128, LC], F32, tag="pmm", name=f"pt_{b}{c}{jc}")
                    for ic in range(NDC):
                        mm(p[:], wbt_tile[:, ic, c, jc * 128:(jc + 1) * 128], sT[ic][:],
                           start=(ic == 0), stop=(ic == NDC - 1))
                    t_ = tring.tile([128, LC], F16, tag="t", name=f"t_{b}{c}{jc}")
                    nc.vector.tensor_scalar_add(
                        t_[:], p[:],
                        f32p[:, COL_VC + c * NJC + jc:COL_VC + c * NJC + jc + 1])
                    tt.append(t_)

                # score_c[x, y] = sum_j t_c^T[j, x] * end^T[j, y] + t1_c[x],
                # accumulated into a [128, 4, LC] bf16 plane group, DMA'd out
                # as one [LC, LC] label plane on the scalar hardware queue.
                planes = oplane.tile([128, NIC, LC], F16, tag="opl",
                                     name=f"opl_{b}{c}")
                last = (b == NB - 1 and c == C - 1)
                for xc in range(NIC):
                    p = pmm.tile([128, LC], F32, tag="pmm", name=f"ps_{b}{c}{xc}")
                    for jc in range(NJC):
                        mm(p[:], tt[jc][:, xc * 128:(xc + 1) * 128], eT[jc][:],
                           start=(jc == 0), stop=(jc == NJC - 1))
                    if last and xc % 2 == 1:
                        nc.vector.tensor_scalar_add(planes[:, xc, :], p[:],
                                                    t1cols[xc][:, c:c + 1])
                    else:
                        nc.scalar.activation(planes[:, xc, :], p[:], AF.Identity,
                                             bias=t1cols[xc][:, c:c + 1], scale=1.0)
                    if last:
                        nc.sync.dma_start(out=out_d[b, c, :, xc, :],
                                          in_=planes[:, xc, :])
                if not last:
                    nc.sync.dma_start(out=out_d[b, c], in_=planes[:])

        # ---- phase-interleaved schedule ----
        # A0+A1 interleaved, then both FFWs, then both biaffines (sT/eT are
        # double-buffered), so the PE stream never stalls on front-end work
        # mid-kernel.
        xT = attention_pair()
        se0 = ffw(0, xT[0])
        se1 = ffw(1, xT[1])
        biaffine(0, *se0)
        biaffine(1, *se1)


_PROGRAM_CACHE = {}


def _get_program():
    if "nc" not in _PROGRAM_CACHE:
        _PROGRAM_CACHE["nc"] = _build_program()
    return _PROGRAM_CACHE["nc"]


def _pack_host_inputs(w_sim, W1s, b1s, W2s, b2s, W1e, b1e, W2e, b2e, Wb):
    """Build the shared (replicated) input arrays from the raw weights."""
    import ml_dtypes
    f32, f16 = np.float32, ml_dtypes.bfloat16
    w1, w2, w3 = [np.asarray(w_sim[k * H:(k + 1) * H], f32) for k in range(3)]

    f32p = np.zeros((128, NF32), f32)
    for hc in range(NHC):
        f32p[:, COL_W3 + hc] = w3[hc * 128:(hc + 1) * 128]
    for fc in range(NFC):
        f32p[:, COL_B1S + fc] = b1s[fc * 128:(fc + 1) * 128]
        f32p[:, COL_B1E + fc] = b1e[fc * 128:(fc + 1) * 128]
    for dc in range(NDC):
        f32p[:, COL_B2S + dc] = b2s[dc * 128:(dc + 1) * 128]
        f32p[:, COL_B2E + dc] = b2e[dc * 128:(dc + 1) * 128]
    for c in range(C):
        for jc in range(NJC):
            f32p[:, COL_VC + c * NJC + jc] = Wb[D, c, jc * 128:(jc + 1) * 128]

    bf16p = np.zeros((128, NBF16), f32)
    bf16p[:, OFF_ONES:OFF_ONES + 512] = 1.0
    for hc in range(NHC):
        bf16p[:, OFF_COLW + 2 * hc] = w1[hc * 128:(hc + 1) * 128]
        bf16p[:, OFF_COLW + 5 + 2 * hc] = w2[hc * 128:(hc + 1) * 128]
    bf16p[:, OFF_Z] = 1.0
    bf16p[:, OFF_Z + 2] = 1.0
    bf16p[:, OFF_IDENT:OFF_IDENT + 128] = np.eye(128, dtype=f32)
    for dc in range(NDC):
        for c in range(C):
            bf16p[:, OFF_UPACK + dc * 4 + c] = Wb[dc * 128:(dc + 1) * 128, c, D]
    bf16p[:, OFF_WROW4:OFF_WROW4 + C] = Wb[D, :, D][None, :]

    def pmaj(a, nchunk):
        # [nchunk*128, F...] -> [128, nchunk, F...] (SBUF-tile layout)
        return np.ascontiguousarray(
            a.reshape((nchunk, 128) + a.shape[1:]).swapaxes(0, 1))

    return {
        "f32pack": f32p,
        "bf16pack": bf16p.astype(f16),
        "wb": pmaj(Wb[:D].astype(f16), NDC),
        "W1s": pmaj(W1s.astype(f16), NDC),
        "W2s": pmaj(W2s.astype(f16), NFC),
        "W1e": pmaj(W1e.astype(f16), NDC),
        "W2e": pmaj(W2e.astype(f16), NFC),
    }


def kernel(ctx_emb, ques_emb, w_sim, W1s, b1s, W2s, b2s, W1e, b1e, W2e, b2e, Wb,
           _trace=False, _tmpdir=None):
    from concourse.bass_utils import run_bass_kernel_spmd
    import ml_dtypes

    # accept jax/np arrays of any layout
    (ctx_emb, ques_emb, w_sim, W1s, b1s, W2s, b2s, W1e, b1e, W2e, b2e, Wb) = (
        np.asarray(a, dtype=np.float32)
        for a in (ctx_emb, ques_emb, w_sim, W1s, b1s, W2s, b2s, W1e, b1e, W2e,
                  b2e, Wb))

    nc = _get_program()
    shared = _pack_host_inputs(w_sim, W1s, b1s, W2s, b2s, W1e, b1e, W2e, b2e, Wb)
    ctx16 = ctx_emb.astype(ml_dtypes.bfloat16)
    ques16 = np.ascontiguousarray(ques_emb.astype(ml_dtypes.bfloat16))
    # p-major repacks matching the SBUF tile layouts ([.., 128, chunk, free])
    ctxp = np.ascontiguousarray(
        ctx16.reshape(B, NIC, 128, H).swapaxes(1, 2))            # [B,128,4,H]
    ctxTp = np.ascontiguousarray(
        ctx16.transpose(0, 2, 1).reshape(B, NHC, 128, LC).swapaxes(1, 2))
    quesTp = np.ascontiguousarray(
        ques16.transpose(0, 2, 1).reshape(B, NHC, 128, LQ).swapaxes(1, 2))
    in_maps = []
    for core in range(N_CORES):
        sl = slice(core * NB, (core + 1) * NB)
        in_maps.append({"ctx": ctxp[sl], "ques": ques16[sl],
                        "ctxT": ctxTp[sl], "quesT": quesTp[sl], **shared})

    kw = {}
    if _trace:
        kw = {"trace": True, "tmpdir": _tmpdir}
    res = run_bass_kernel_spmd(nc, in_maps, list(range(N_CORES)), **kw)
    # device layout is [NB, C, 128, NIC, LC] bf16 (p-major planes);
    # upcast + unpermute to [B, LC, LC, C] on the host.
    outs = []
    for i in range(N_CORES):
        o = np.asarray(res.results[i]["out"])  # [NB, C, 128, NIC, LC]
        o = o.astype(np.float32).transpose(0, 3, 2, 4, 1)  # [NB, NIC, 128, LC, C]
        outs.append(o.reshape(NB, LC, LC, C))
    out = np.ascontiguousarray(np.concatenate(outs, axis=0))
    if _trace:
        return out, res
    return out


# revision 28
# speedup vs baseline: 1.1836x; 1.1836x over previous
"""Trainium2 Bass kernel for nn_BiaffineNER (BiDAF attention + FFW + biaffine scorer).

Contract: kernel(**inputs) takes the FULL unsharded inputs (numpy) and returns
the FULL [16, 512, 512, 3] float32 output. Internally shards data-parallel over
the batch axis across 8 NeuronCores (2 batch elements per core), runs one SPMD
Bass/Tile program on all cores, and concatenates the per-core outputs.

Math per batch element b (LC=512, LQ=64, H=256, D=4H=1024, DFF=512, C=3):
  sim  = (ctx@w1)[:,None] + (ques@w2)[None,:] + (ctx*w3)@ques.T      [LC,LQ]
  a    = softmax_j(sim); c2q = a @ ques                              [LC,H]
  bwt  = softmax_i(max_j sim); q2c = bwt @ ctx                       [H]
  x    = [ctx, c2q, ctx*c2q, ctx*q2c]                                [LC,D]
  start= relu(x@W1s+b1s)@W2s+b2s ; end likewise                      [LC,D]
  out[x,y,c] = [start,1][x] . Wb[:,c,:] . [end,1][y]                 [LC,LC,C]

Design notes:
- Activations kept transposed on-chip ([feature-part, token-free]) so the
  contraction dim always sits on SBUF partitions; ctx^T/ques^T come in
  host-pretransposed.
- All matmul operands are bfloat16: the PE runs bf16 at 1 cycle/row (full
  rate; fp32r pays a serialized 4-byte LDWEIGHTS ~176ns/matmul, and IEEE fp16
  measures 2 cycles/row on real HW).  End-to-end rel err ~6e-3 (tolerance
  2e-2); accumulation stays fp32 in PSUM.
- All weights (FFW + the 12.6MB Wb) are loaded once in bf16 and stay resident
  in SBUF for both batch elements: HBM read drops ~44MB -> ~12MB per core.
- dma_start issue cost is ~0.6-0.75us of engine time each, so DMAs are merged
  aggressively (host arrays pre-packed p-major so merged DMAs stay contiguous):
  2 constant packs, 1 DMA per FFW weight matrix, 1 DMA for all of Wb, 4 DMAs
  per batch of inputs, one output DMA per (batch, label) plane group.
- DMA queue plan: each hw queue (scalar/Activation, sync/SP) carries one
  batch's critical inputs FIRST; all 10.3MB of bulk weights then ride the
  sync queue alone, ordered by first use (the Activation queue only sustains
  ~90GB/s while sync's is active; sync alone does ~290GB/s).  Output planes
  also leave via sync.  gpsimd's software queue (~43GB/s) carries nothing.
- The output leaves as bf16 [C, LC, LC] planes (host transposes/upcasts for
  free), so the kernel tail is one 512KB DMA, not a whole batch element.
- Softmaxes skip max-subtraction (|sim| < ~8 for this data distribution), which
  turns the partition-axis softmax over i into tiny matmul reductions.
- The two batch elements' attention front-ends are instruction-interleaved
  (independent dependency chains hide each other's latency), then
  F0 F1 B0 B1, with a ~3us dependency-free warm-up matmul burst up front so
  the PE HAM clock gate opens before the first dense phase.
"""

import sys

if "/opt/trn_rl_repo" not in sys.path and "/root/.axon_site/_ro/trn_rl_repo" not in sys.path:
    sys.path.insert(0, "/opt/trn_rl_repo")

import numpy as np

import concourse.bass as bass
import concourse.tile as tile
from concourse import bacc, mybir

F32 = mybir.dt.float32
F16 = mybir.dt.bfloat16
AF = mybir.ActivationFunctionType
ALU = mybir.AluOpType
AX = mybir.AxisListType

N_CORES = 8
B, LC, LQ, H = 16, 512, 64, 256
NB = B // N_CORES          # batch elements per core
D, DFF, C = 4 * H, 512, 3
NIC = LC // 128            # 4  i/x chunks
NHC = H // 128             # 2  h chunks
NDC = D // 128             # 8  d chunks
NFC = DFF // 128           # 4  f chunks
NJC = 8                    # j chunks (first 1024 of 1025)

# f32 constant pack column layout [128, 54]
COL_W3 = 0                 # 2 cols (w3 chunks, per-partition scalars)
COL_B1S, COL_B2S = 2, 6    # 4 + 8
COL_B1E, COL_B2E = 14, 18  # 4 + 8
COL_VC = 26                # 24 cols: vcols[c*NJC + jc]
NF32 = 50

# bf16 constant pack column layout [128, 687]
OFF_ONES = 0               # 512 cols of 1.0 (ones_row row 0; ones2 any 2 cols)
OFF_COLW = 512             # 8 cols: [w1_0, 0, w1_1, 0, 0, w2_0, 0, w2_1]
OFF_IDENT = 520            # 128 cols identity
OFF_UPACK = 648            # 32 cols: upack[dc*4 + c] = Wb[dc*128+p, c, D]
OFF_WROW4 = 680            # 4 cols: Wb[D, c, D] broadcast down partitions
OFF_Z = 684                # 3 cols [1, 0, 1]: [z:z+2]=[1,0], [z+1:z+3]=[0,1]
NBF16 = 687


def _build_program():
    nc = bacc.Bacc("TRN2", target_bir_lowering=False, debug=False,
                   num_devices=N_CORES)

    ctx_d = nc.dram_tensor("ctx", [NB, 128, NIC, H], F16, kind="ExternalInput").ap()
    ques_d = nc.dram_tensor("ques", [NB, LQ, H], F16, kind="ExternalInput").ap()
    ctxT_d = nc.dram_tensor("ctxT", [NB, 128, NHC, LC], F16, kind="ExternalInput").ap()
    quesT_d = nc.dram_tensor("quesT", [NB, 128, NHC, LQ], F16, kind="ExternalInput").ap()
    f32p_d = nc.dram_tensor("f32pack", [128, NF32], F32, kind="ExternalInput").ap()
    bf16p_d = nc.dram_tensor("bf16pack", [128, NBF16], F16, kind="ExternalInput").ap()
    wb_d = nc.dram_tensor("wb", [128, NDC, C, D + 1], F16, kind="ExternalInput").ap()
    w1s_d = nc.dram_tensor("W1s", [128, NDC, DFF], F16, kind="ExternalInput").ap()
    w2s_d = nc.dram_tensor("W2s", [128, NFC, D], F16, kind="ExternalInput").ap()
    w1e_d = nc.dram_tensor("W1e", [128, NDC, DFF], F16, kind="ExternalInput").ap()
    w2e_d = nc.dram_tensor("W2e", [128, NFC, D], F16, kind="ExternalInput").ap()
    out_d = nc.dram_tensor("out", [NB, C, 128, NIC, LC], F16, kind="ExternalOutput").ap()

    with tile.TileContext(nc) as tc:
        _trace_kernel(nc, tc, ctx_d, ques_d, ctxT_d, quesT_d, f32p_d, bf16p_d,
                      wb_d, (w1s_d, w2s_d), (w1e_d, w2e_d), out_d)
    nc.compile()
    return nc


def _trace_kernel(nc, tc, ctx_d, ques_d, ctxT_d, quesT_d, f32p_d, bf16p_d,
                  wb_d, ws_d, we_d, out_d):
    import contextlib
    est = contextlib.ExitStack()
    with est:
        const = est.enter_context(tc.tile_pool(name="const", bufs=1))
        attn = est.enter_context(tc.tile_pool(name="attn", bufs=1))
        wres = est.enter_context(tc.tile_pool(name="wres", bufs=1))
        tring = est.enter_context(tc.tile_pool(name="tring", bufs=9))
        acts = est.enter_context(tc.tile_pool(name="acts", bufs=1))
        oplane = est.enter_context(tc.tile_pool(name="oplane", bufs=3))
        cols = est.enter_context(tc.tile_pool(name="cols", bufs=2))
        pmm = est.enter_context(tc.tile_pool(name="pmm", bufs=3, space="PSUM"))
        pffw = est.enter_context(tc.tile_pool(name="pffw", bufs=3, space="PSUM"))
        ptiny = est.enter_context(tc.tile_pool(name="ptiny", bufs=2, space="PSUM"))
        pat = ptiny

        def mm(out, lhsT, rhs, start, stop):
            nc.tensor.matmul(out, lhsT, rhs, start=start, stop=stop)

        # HAM warm-up: ~3us of dependency-free PE activity (plain fp32, fed by
        # a memset tile) so the clock gate opens before the first dense phase.
        ones2_f = const.tile([128, 2], F32, tag="ones2_f")
        nc.vector.memset(ones2_f[:], 1.0)
        p_warm = pmm.tile([128, 512], F32, tag="pmm")
        for wi in range(60):
            nc.tensor.matmul(p_warm[0:2, 0:2], ones2_f[:], ones2_f[:],
                             start=(wi == 0), stop=(wi == 59))

        # ---- DMA plan: each hw queue carries one batch's critical inputs
        # FIRST (plus one const pack), then its share of the bulk weights.
        # Criticals on both queues drain in parallel at full HBM rate before
        # any bulk weight competes for bandwidth.
        f32p = const.tile([128, NF32], F32, tag="f32p")
        nc.scalar.dma_start(out=f32p[:], in_=f32p_d[:])
        bf16p = const.tile([128, NBF16], F16, tag="bf16p")
        nc.sync.dma_start(out=bf16p[:], in_=bf16p_d[:])

        colw = bf16p[:, OFF_COLW:OFF_COLW + 8]
        ident = bf16p[:, OFF_IDENT:OFF_IDENT + 128]
        ones_row = bf16p[0:1, OFF_ONES:OFF_ONES + 512]
        ones2 = bf16p[:, OFF_ONES:OFF_ONES + 2]
        wrow4 = bf16p[0:1, OFF_WROW4:OFF_WROW4 + 4]

        quesT_sb, ctxT_sb, ques_sb, ctx_sb = {}, {}, {}, {}
        for b, eng in ((0, nc.scalar), (1, nc.sync)):
            t_ = attn.tile([128, NHC, LQ], F16, tag=f"quesT_{b}", name=f"quesT_{b}")
            eng.dma_start(out=t_[:], in_=quesT_d[b])
            quesT_sb[b] = [t_[:, hc, :] for hc in range(NHC)]
            t_ = attn.tile([128, NHC, LC], F16, tag=f"ctxT_{b}", name=f"ctxT_{b}")
            eng.dma_start(out=t_[:], in_=ctxT_d[b])
            ctxT_sb[b] = [t_[:, hc, :] for hc in range(NHC)]
            q_ = attn.tile([LQ, H], F16, tag=f"ques_{b}", name=f"ques_{b}")
            eng.dma_start(out=q_[:], in_=ques_d[b, :, :])
            ques_sb[b] = q_
            t_ = attn.tile([128, NIC, H], F16, tag=f"ctx_{b}", name=f"ctx_{b}")
            eng.dma_start(out=t_[:], in_=ctx_d[b])
            ctx_sb[b] = [t_[:, ic, :] for ic in range(NIC)]

        # bulk weights, after the criticals, ALL on the sync queue ordered by
        # first use (w1s ~20us ... Wb ~80us).  The scalar (Activation) hw
        # queue only gets ~90GB/s when sync's queue is active, so bulk on it
        # arrives late; sync alone moves 10.3MB well before each deadline.
        wtiles = {}
        for lname, (w1_d, w2_d) in (("s", ws_d), ("e", we_d)):
            w1t = wres.tile([128, NDC, DFF], F16, tag=f"w1{lname}", name=f"w1{lname}")
            nc.sync.dma_start(out=w1t[:], in_=w1_d[:])
            w2t = wres.tile([128, NFC, D], F16, tag=f"w2{lname}", name=f"w2{lname}")
            nc.sync.dma_start(out=w2t[:], in_=w2_d[:])
            wtiles[lname] = (w1t, w2t)
        wbt_tile = wres.tile([128, NDC, C, D + 1], F16, tag="wb")
        nc.sync.dma_start(out=wbt_tile[:], in_=wb_d[:])

        def attention_pair():
            """Both batches' attention, instruction-interleaved stage by stage.
            Returns {b: xT chunk list} (8 tiles [128, LC] bf16 each)."""
            BS = (0, 1)
            quesT, ctxT = quesT_sb, ctxT_sb

            # (ctx*w3)^T
            ctxw3T = {b: [] for b in BS}
            for b in BS:
                for hc in range(NHC):
                    t_ = attn.tile([128, LC], F16, tag=f"ctxw3T{hc}_{b}")
                    nc.vector.tensor_scalar_mul(
                        t_[:], ctxT[b][hc],
                        f32p[:, COL_W3 + hc:COL_W3 + hc + 1])
                    ctxw3T[b].append(t_)

            # Stacked rank-2 tiles, built wholly in PSUM via zero-padded
            # weight columns + a rank-1 ones term: q2ones = [ones; ques@w2],
            # c1ones = [ctx@w1; ones].  Both broadcast terms of sim are then
            # ONE matmul: c1ones[:,isl].T @ q2ones = c1[i]*1 + 1*q2[j].
            o10 = bf16p[0:1, OFF_Z:OFF_Z + 2]
            o01 = bf16p[0:1, OFF_Z + 1:OFF_Z + 3]
            q2ones, c1ones = {}, {}
            for b in BS:
                p_q2r = pat.tile([2, LQ], F32, tag="pt", name=f"pq2r_{b}")
                for hc in range(NHC):
                    mm(p_q2r[:], colw[:, 4 + 2 * hc:6 + 2 * hc], quesT[b][hc],
                       start=(hc == 0), stop=False)
                mm(p_q2r[:], o10, ones_row[:, 0:LQ], start=False, stop=True)
                q2ones[b] = cols.tile([2, LQ], F16, tag="q2row", name=f"q2row_{b}")
                nc.vector.tensor_copy(q2ones[b][:], p_q2r[:])
            for b in BS:
                p_c1r = pffw.tile([2, LC], F32, tag="pf", name=f"pc1r_{b}")
                for hc in range(NHC):
                    mm(p_c1r[:], colw[:, 2 * hc:2 * hc + 2], ctxT[b][hc],
                       start=(hc == 0), stop=False)
                mm(p_c1r[:], o01, ones_row[:, 0:LC], start=False, stop=True)
                c1ones[b] = cols.tile([2, LC], F16, tag="c1row", name=f"c1row_{b}")
                nc.vector.tensor_copy(c1ones[b][:], p_c1r[:])

            ucols = {b: attn.tile([128, NIC + 2], F16, tag=f"ucols_{b}",
                                  name=f"ucols_{b}") for b in BS}
            a_n = {b: [] for b in BS}
            for ic in range(NIC):
                icsl = slice(ic * 128, (ic + 1) * 128)
                for b in BS:
                    p_sim = pmm.tile([128, LQ], F32, tag="pmm", name=f"psim_{b}{ic}")
                    for hc in range(NHC):
                        mm(p_sim[:], ctxw3T[b][hc][:, icsl], quesT[b][hc],
                           start=(hc == 0), stop=False)
                    mm(p_sim[:], c1ones[b][:, icsl], q2ones[b][:],
                       start=False, stop=True)

                    a_un = attn.tile([128, LQ], F32, tag=f"aun{ic}_{b}")
                    nc.scalar.activation(a_un[:], p_sim[:], AF.Exp)
                    ssum = cols.tile([128, 1], F32, tag="ssum", name=f"ssum_{b}{ic}")
                    nc.vector.reduce_sum(out=ssum[:], in_=a_un[:], axis=AX.X)
                    srec = cols.tile([128, 1], F32, tag="srec", name=f"srec_{b}{ic}")
                    nc.vector.reciprocal(srec[:], ssum[:])
                    nc.vector.reduce_max(out=ucols[b][:, ic:ic + 1], in_=a_un[:], axis=AX.X)
                    t_ = attn.tile([128, LQ], F16, tag=f"an{ic}_{b}")
                    nc.vector.tensor_scalar_mul(t_[:], a_un[:], srec[:])
                    a_n[b].append(t_)

            # a^T [j-part, i-free]
            aT = {b: attn.tile([LQ, LC], F16, tag=f"aT_{b}", name=f"aT_{b}")
                  for b in BS}
            for b in BS:
                for ic in range(NIC):
                    p = pffw.tile([LQ, 128], F16, tag="pf", name=f"paT_{b}{ic}")
                    nc.tensor.transpose(p[:], a_n[b][ic][:], ident)
                    nc.scalar.activation(aT[b][:, ic * 128:(ic + 1) * 128], p[:], AF.Copy)

            # softmax-over-i weights: denominator + broadcast of 1/den
            invb = {}
            for b in BS:
                ucol1 = cols.tile([128, 1], F16, tag="ucol1", name=f"ucol1_{b}")
                with nc.allow_low_precision(reason="4-term bf16 softmax-denominator partial sum"):
                    nc.vector.reduce_sum(out=ucol1[:], in_=ucols[b][:, 0:NIC], axis=AX.X)
                p_den = pat.tile([1, 2], F32, tag="pt", name=f"pden_{b}")
                mm(p_den[:], ucol1[:], ones2, start=True, stop=True)
                inv2f = cols.tile([1, 2], F32, tag="inv2f", name=f"inv2f_{b}")
                nc.vector.reciprocal(inv2f[:], p_den[:])
                inv2 = cols.tile([1, 2], F16, tag="inv2", name=f"inv2_{b}")
                nc.scalar.activation(inv2[:], inv2f[:], AF.Copy)
                p_bc = pat.tile([128, 2], F32, tag="pt", name=f"pbc_{b}")
                mm(p_bc[:], ones_row[:, 0:128], inv2[:], start=True, stop=True)
                invb[b] = cols.tile([128, 1], F32, tag="invb", name=f"invb_{b}")
                nc.scalar.activation(invb[b][:], p_bc[:, 0:1], AF.Copy)

            q2cc = {b: [] for b in BS}
            for b in BS:
                for hs in range(NHC):
                    p_q2c = pat.tile([128, 2], F32, tag="pt", name=f"pq2c_{b}{hs}")
                    for ic in range(NIC):
                        mm(p_q2c[:], ctx_sb[b][ic][:, hs * 128:(hs + 1) * 128],
                           ucols[b][:, ic:ic + 2], start=(ic == 0), stop=(ic == NIC - 1))
                    t_ = cols.tile([128, 1], F32, tag=f"q2cc{hs}", name=f"q2cc_{b}{hs}")
                    nc.vector.tensor_mul(t_[:], p_q2c[:, 0:1], invb[b][:])
                    q2cc[b].append(t_)

            # x^T chunks: 0-1 ctx^T, 2-3 c2q^T, 4-5 (ctx*c2q)^T, 6-7 (ctx*q2c)^T
            xT = {}
            for b in BS:
                xT[b] = [ctxT[b][0], ctxT[b][1]]
                for hs in range(NHC):
                    p_c2q = pffw.tile([128, LC], F32, tag="pf", name=f"pc2q_{b}{hs}")
                    mm(p_c2q[:], ques_sb[b][:, hs * 128:(hs + 1) * 128], aT[b][:],
                       start=True, stop=True)
                    t_ = acts.tile([128, LC], F16, tag=f"xT{2 + hs}_{b}")
                    nc.scalar.activation(t_[:], p_c2q[:], AF.Copy)
                    xT[b].append(t_)
                for hc in range(NHC):
                    t_ = acts.tile([128, LC], F16, tag=f"xT{4 + hc}_{b}")
                    nc.vector.tensor_mul(t_[:], ctxT[b][hc], xT[b][2 + hc][:])
                    xT[b].append(t_)
                for hc in range(NHC):
                    t_ = acts.tile([128, LC], F16, tag=f"xT{6 + hc}_{b}")
                    nc.vector.tensor_scalar_mul(t_[:], ctxT[b][hc], q2cc[b][hc][:])
                    xT[b].append(t_)
            return xT

        def ffw(b, xT):
            sT, eT = [], []
            for lname, colb1, colb2, dst in (
                ("s", COL_B1S, COL_B2S, sT),
                ("e", COL_B1E, COL_B2E, eT),
            ):
                w1t, w2t = wtiles[lname]
                h1 = []
                dc_order = [0, 1, 6, 7, 2, 3, 4, 5]
                for fc in range(NFC):
                    p = pffw.tile([128, LC], F32, tag="pf", name=f"ph1{lname}_{b}{fc}")
                    for k, dc in enumerate(dc_order):
                        mm(p[:], w1t[:, dc, fc * 128:(fc + 1) * 128], xT[dc],
                           start=(k == 0), stop=(k == NDC - 1))
                    t_ = acts.tile([128, LC], F16, tag=f"h1{fc}",
                                   name=f"h1{lname}{fc}_{b}")
                    nc.vector.tensor_scalar(
                        out=t_[:], in0=p[:],
                        scalar1=f32p[:, colb1 + fc:colb1 + fc + 1],
                        scalar2=0.0, op0=ALU.add, op1=ALU.max)
                    h1.append(t_)
                for dc in range(NDC):
                    p = pffw.tile([128, LC], F32, tag="pf", name=f"po{lname}_{b}{dc}")
                    for fc in range(NFC):
                        mm(p[:], w2t[:, fc, dc * 128:(dc + 1) * 128], h1[fc][:],
                           start=(fc == 0), stop=(fc == NFC - 1))
                    t_ = acts.tile([128, LC], F16, tag=f"{lname}T{dc}", bufs=2,
                                   name=f"{lname}T{dc}_{b}")
                    nc.scalar.activation(
                        t_[:], p[:], AF.Identity,
                        bias=f32p[:, colb2 + dc:colb2 + dc + 1],
                        scale=1.0)
                    dst.append(t_)
            return sT, eT

        def biaffine(b, sT, eT):
            # t1 rows for all three labels in one group:
            # t1[c, x] = sum_i start^T[i, x] * Wb[i, c, D]  + Wb[D, c, D]
            p_t14 = pffw.tile([4, LC], F32, tag="pf", name=f"pt14_{b}")
            for ic in range(NDC):
                mm(p_t14[:], bf16p[:, OFF_UPACK + ic * 4:OFF_UPACK + ic * 4 + 4],
                   sT[ic][:], start=(ic == 0), stop=False)
            mm(p_t14[:], wrow4, ones_row, start=False, stop=True)
            t14 = cols.tile([4, LC], F16, tag="t14", name=f"t14_{b}")
            nc.scalar.activation(t14[:], p_t14[:], AF.Copy)
            t1cols = []
            for xc in range(NIC):
                p = ptiny.tile([128, 4], F16, tag="pt", name=f"pt1c_{b}{xc}")
                nc.tensor.transpose(p[:], t14[:, xc * 128:(xc + 1) * 128],
                                    ident[0:4, 0:4])
                tsb = cols.tile([128, 4], F32, tag=f"t1c{xc}", name=f"t1c{xc}_{b}")
                nc.vector.tensor_copy(tsb[:], p[:])
                t1cols.append(tsb)

            for c in range(C):
                # t_c^T[j, x] = sum_i Wb[i,c,j] * start^T[i, x]  (+ v_c[j])
                tt = []
                for jc in range(NJC):
                    p = pmm.tile([128, LC], F32, tag="pmm", name=f"pt_{b}{c}{jc}")
                    for ic in range(NDC):
                        mm(p[:], wbt_tile[:, ic, c, jc * 128:(jc + 1) * 128], sT[ic][:],
                           start=(ic == 0), stop=(ic == NDC - 1))
                    t_ = tring.tile([128, LC], F16, tag="t", name=f"t_{b}{c}{jc}")
                    nc.vector.tensor_scalar_add(
                        t_[:], p[:],
                        f32p[:, COL_VC + c * NJC + jc:COL_VC + c * NJC + jc + 1])
                    tt.append(t_)

                # score_c[x, y] = sum_j t_c^T[j, x] * end^T[j, y] + t1_c[x],
                # accumulated into a [128, 4, LC] bf16 plane group, DMA'd out
                # as one [LC, LC] label plane on the scalar hardware queue.
                planes = oplane.tile([128, NIC, LC], F16, tag="opl",
                                     name=f"opl_{b}{c}")
                last = (b == NB - 1 and c == C - 1)
                for xc in range(NIC):
                    p = pmm.tile([128, LC], F32, tag="pmm", name=f"ps_{b}{c}{xc}")
                    for jc in range(NJC):
                        mm(p[:], tt[jc][:, xc * 128:(xc + 1) * 128], eT[jc][:],
                           start=(jc == 0), stop=(jc == NJC - 1))
                    if last and xc % 2 == 1:
                        nc.vector.tensor_scalar_add(planes[:, xc, :], p[:],
                                                    t1cols[xc][:, c:c + 1])
                    else:
                        nc.scalar.activation(planes[:, xc, :], p[:], AF.Identity,
                                             bias=t1cols[xc][:, c:c + 1], scale=1.0)
                    if last:
                        nc.sync.dma_start(out=out_d[b, c, :, xc, :],
                                          in_=planes[:, xc, :])
                if not last:
                    nc.sync.dma_start(out=out_d[b, c], in_=planes[:])

        # ---- phase-interleaved schedule ----
        # A0+A1 interleaved, then both FFWs, then both biaffines (sT/eT are
        # double-buffered), so the PE stream never stalls on front-end work
        # mid-kernel.
        xT = attention_pair()
        se0 = ffw(0, xT[0])
        se1 = ffw(1, xT[1])
        biaffine(0, *se0)
        biaffine(1, *se1)


_PROGRAM_CACHE = {}


def _get_program():
    if "nc" not in _PROGRAM_CACHE:
        _PROGRAM_CACHE["nc"] = _build_program()
    return _PROGRAM_CACHE["nc"]


def _pack_host_inputs(w_sim, W1s, b1s, W2s, b2s, W1e, b1e, W2e, b2e, Wb):
    """Build the shared (replicated) input arrays from the raw weights."""
    import ml_dtypes
    f32, f16 = np.float32, ml_dtypes.bfloat16
    w1, w2, w3 = [np.asarray(w_sim[k * H:(k + 1) * H], f32) for k in range(3)]

    f32p = np.zeros((128, NF32), f32)
    for hc in range(NHC):
        f32p[:, COL_W3 + hc] = w3[hc * 128:(hc + 1) * 128]
    for fc in range(NFC):
        f32p[:, COL_B1S + fc] = b1s[fc * 128:(fc + 1) * 128]
        f32p[:, COL_B1E + fc] = b1e[fc * 128:(fc + 1) * 128]
    for dc in range(NDC):
        f32p[:, COL_B2S + dc] = b2s[dc * 128:(dc + 1) * 128]
        f32p[:, COL_B2E + dc] = b2e[dc * 128:(dc + 1) * 128]
    for c in range(C):
        for jc in range(NJC):
            f32p[:, COL_VC + c * NJC + jc] = Wb[D, c, jc * 128:(jc + 1) * 128]

    bf16p = np.zeros((128, NBF16), f32)
    bf16p[:, OFF_ONES:OFF_ONES + 512] = 1.0
    for hc in range(NHC):
        bf16p[:, OFF_COLW + 2 * hc] = w1[hc * 128:(hc + 1) * 128]
        bf16p[:, OFF_COLW + 5 + 2 * hc] = w2[hc * 128:(hc + 1) * 128]
    bf16p[:, OFF_Z] = 1.0
    bf16p[:, OFF_Z + 2] = 1.0
    bf16p[:, OFF_IDENT:OFF_IDENT + 128] = np.eye(128, dtype=f32)
    for dc in range(NDC):
        for c in range(C):
            bf16p[:, OFF_UPACK + dc * 4 + c] = Wb[dc * 128:(dc + 1) * 128, c, D]
    bf16p[:, OFF_WROW4:OFF_WROW4 + C] = Wb[D, :, D][None, :]

    def pmaj(a, nchunk):
        # [nchunk*128, F...] -> [128, nchunk, F...] (SBUF-tile layout)
        return np.ascontiguousarray(
            a.reshape((nchunk, 128) + a.shape[1:]).swapaxes(0, 1))

    return {
        "f32pack": f32p,
        "bf16pack": bf16p.astype(f16),
        "wb": pmaj(Wb[:D].astype(f16), NDC),
        "W1s": pmaj(W1s.astype(f16), NDC),
        "W2s": pmaj(W2s.astype(f16), NFC),
        "W1e": pmaj(W1e.astype(f16), NDC),
        "W2e": pmaj(W2e.astype(f16), NFC),
    }


def kernel(ctx_emb, ques_emb, w_sim, W1s, b1s, W2s, b2s, W1e, b1e, W2e, b2e, Wb,
           _trace=False, _tmpdir=None):
    from concourse.bass_utils import run_bass_kernel_spmd
    import ml_dtypes

    # accept jax/np arrays of any layout
    (ctx_emb, ques_emb, w_sim, W1s, b1s, W2s, b2s, W1e, b1e, W2e, b2e, Wb) = (
        np.asarray(a, dtype=np.float32)
        for a in (ctx_emb, ques_emb, w_sim, W1s, b1s, W2s, b2s, W1e, b1e, W2e,
                  b2e, Wb))

    nc = _get_program()
    shared = _pack_host_inputs(w_sim, W1s, b1s, W2s, b2s, W1e, b1e, W2e, b2e, Wb)
    ctx16 = ctx_emb.astype(ml_dtypes.bfloat16)
    ques16 = np.ascontiguousarray(ques_emb.astype(ml_dtypes.bfloat16))
    # p-major repacks matching the SBUF tile layouts ([.., 128, chunk, free])
    ctxp = np.ascontiguousarray(
        ctx16.reshape(B, NIC, 128, H).swapaxes(1, 2))            # [B,128,4,H]
    ctxTp = np.ascontiguousarray(
        ctx16.transpose(0, 2, 1).reshape(B, NHC, 128, LC).swapaxes(1, 2))
    quesTp = np.ascontiguousarray(
        ques16.transpose(0, 2, 1).reshape(B, NHC, 128, LQ).swapaxes(1, 2))
    in_maps = []
    for core in range(N_CORES):
        sl = slice(core * NB, (core + 1) * NB)
        in_maps.append({"ctx": ctxp[sl], "ques": ques16[sl],
                        "ctxT": ctxTp[sl], "quesT": quesTp[sl], **shared})

    kw = {}
    if _trace:
        kw = {"trace": True, "tmpdir": _tmpdir}
    res = run_bass_kernel_spmd(nc, in_maps, list(range(N_CORES)), **kw)
    # device layout is [NB, C, 128, NIC, LC] bf16 (p-major planes);
    # upcast + unpermute to [B, LC, LC, C] on the host.
    outs = []
    for i in range(N_CORES):
        o = np.asarray(res.results[i]["out"])  # [NB, C, 128, NIC, LC]
        o = o.astype(np.float32).transpose(0, 3, 2, 4, 1)  # [NB, NIC, 128, LC, C]
        outs.append(o.reshape(NB, LC, LC, C))
    out = np.ascontiguousarray(np.concatenate(outs, axis=0))
    if _trace:
        return out, res
    return out


# revision 29
# speedup vs baseline: 1.1883x; 1.0040x over previous
"""Trainium2 Bass kernel for nn_BiaffineNER (BiDAF attention + FFW + biaffine scorer).

Contract: kernel(**inputs) takes the FULL unsharded inputs (numpy) and returns
the FULL [16, 512, 512, 3] float32 output. Internally shards data-parallel over
the batch axis across 8 NeuronCores (2 batch elements per core), runs one SPMD
Bass/Tile program on all cores, and concatenates the per-core outputs.

Math per batch element b (LC=512, LQ=64, H=256, D=4H=1024, DFF=512, C=3):
  sim  = (ctx@w1)[:,None] + (ques@w2)[None,:] + (ctx*w3)@ques.T      [LC,LQ]
  a    = softmax_j(sim); c2q = a @ ques                              [LC,H]
  bwt  = softmax_i(max_j sim); q2c = bwt @ ctx                       [H]
  x    = [ctx, c2q, ctx*c2q, ctx*q2c]                                [LC,D]
  start= relu(x@W1s+b1s)@W2s+b2s ; end likewise                      [LC,D]
  out[x,y,c] = [start,1][x] . Wb[:,c,:] . [end,1][y]                 [LC,LC,C]

Design notes:
- Activations kept transposed on-chip ([feature-part, token-free]) so the
  contraction dim always sits on SBUF partitions; ctx^T/ques^T come in
  host-pretransposed.
- All matmul operands are bfloat16: the PE runs bf16 at 1 cycle/row (full
  rate; fp32r pays a serialized 4-byte LDWEIGHTS ~176ns/matmul, and IEEE fp16
  measures 2 cycles/row on real HW).  End-to-end rel err ~6e-3 (tolerance
  2e-2); accumulation stays fp32 in PSUM.
- All weights (FFW + the 12.6MB Wb) are loaded once in bf16 and stay resident
  in SBUF for both batch elements: HBM read drops ~44MB -> ~12MB per core.
- dma_start issue cost is ~0.6-0.75us of engine time each, so DMAs are merged
  aggressively (host arrays pre-packed p-major so merged DMAs stay contiguous):
  2 constant packs, 1 DMA per FFW weight matrix, 1 DMA for all of Wb, 4 DMAs
  per batch of inputs, one output DMA per (batch, label) plane group.
- DMA queue plan: each hw queue (scalar/Activation, sync/SP) carries one
  batch's critical inputs FIRST; all 10.3MB of bulk weights then ride the
  sync queue alone, ordered by first use (the Activation queue only sustains
  ~90GB/s while sync's is active; sync alone does ~290GB/s).  Output planes
  also leave via sync.  gpsimd's software queue (~43GB/s) carries nothing.
- The output leaves as bf16 [C, LC, LC] planes (host transposes/upcasts for
  free), so the kernel tail is one 512KB DMA, not a whole batch element.
- Softmaxes skip max-subtraction (|sim| < ~8 for this data distribution), which
  turns the partition-axis softmax over i into tiny matmul reductions.
- The two batch elements' attention front-ends are instruction-interleaved
  (independent dependency chains hide each other's latency), then
  F0 F1 B0 B1, with a ~3us dependency-free warm-up matmul burst up front so
  the PE HAM clock gate opens before the first dense phase.
"""

import sys

if "/opt/trn_rl_repo" not in sys.path and "/root/.axon_site/_ro/trn_rl_repo" not in sys.path:
    sys.path.insert(0, "/opt/trn_rl_repo")

import numpy as np

import concourse.bass as bass
import concourse.tile as tile
from concourse import bacc, mybir

F32 = mybir.dt.float32
F16 = mybir.dt.bfloat16
AF = mybir.ActivationFunctionType
ALU = mybir.AluOpType
AX = mybir.AxisListType

N_CORES = 8
B, LC, LQ, H = 16, 512, 64, 256
NB = B // N_CORES          # batch elements per core
D, DFF, C = 4 * H, 512, 3
NIC = LC // 128            # 4  i/x chunks
NHC = H // 128             # 2  h chunks
NDC = D // 128             # 8  d chunks
NFC = DFF // 128           # 4  f chunks
NJC = 8                    # j chunks (first 1024 of 1025)

# f32 constant pack column layout [128, 54]
COL_W3 = 0                 # 2 cols (w3 chunks, per-partition scalars)
COL_B1S, COL_B2S = 2, 6    # 4 + 8
COL_B1E, COL_B2E = 14, 18  # 4 + 8
COL_VC = 26                # 24 cols: vcols[c*NJC + jc]
NF32 = 50

# bf16 constant pack column layout [128, 687]
OFF_ONES = 0               # 512 cols of 1.0 (ones_row row 0; ones2 any 2 cols)
OFF_COLW = 512             # 8 cols: [w1_0, 0, w1_1, 0, 0, w2_0, 0, w2_1]
OFF_IDENT = 520            # 128 cols identity
OFF_UPACK = 648            # 32 cols: upack[dc*4 + c] = Wb[dc*128+p, c, D]
OFF_WROW4 = 680            # 4 cols: Wb[D, c, D] broadcast down partitions
OFF_Z = 684                # 3 cols [1, 0, 1]: [z:z+2]=[1,0], [z+1:z+3]=[0,1]
NBF16 = 687


def _build_program():
    nc = bacc.Bacc("TRN2", target_bir_lowering=False, debug=False,
                   num_devices=N_CORES)

    ctx_d = nc.dram_tensor("ctx", [NB, 128, NIC, H], F16, kind="ExternalInput").ap()
    ques_d = nc.dram_tensor("ques", [NB, LQ, H], F16, kind="ExternalInput").ap()
    ctxT_d = nc.dram_tensor("ctxT", [NB, 128, NHC, LC], F16, kind="ExternalInput").ap()
    quesT_d = nc.dram_tensor("quesT", [NB, 128, NHC, LQ], F16, kind="ExternalInput").ap()
    f32p_d = nc.dram_tensor("f32pack", [128, NF32], F32, kind="ExternalInput").ap()
    bf16p_d = nc.dram_tensor("bf16pack", [128, NBF16], F16, kind="ExternalInput").ap()
    wb_d = nc.dram_tensor("wb", [128, NDC, C, D + 1], F16, kind="ExternalInput").ap()
    w1s_d = nc.dram_tensor("W1s", [128, NDC, DFF], F16, kind="ExternalInput").ap()
    w2s_d = nc.dram_tensor("W2s", [128, NFC, D], F16, kind="ExternalInput").ap()
    w1e_d = nc.dram_tensor("W1e", [128, NDC, DFF], F16, kind="ExternalInput").ap()
    w2e_d = nc.dram_tensor("W2e", [128, NFC, D], F16, kind="ExternalInput").ap()
    out_d = nc.dram_tensor("out", [NB, C, 128, NIC, LC], F16, kind="ExternalOutput").ap()

    with tile.TileContext(nc) as tc:
        _trace_kernel(nc, tc, ctx_d, ques_d, ctxT_d, quesT_d, f32p_d, bf16p_d,
                      wb_d, (w1s_d, w2s_d), (w1e_d, w2e_d), out_d)
    nc.compile()
    return nc


def _trace_kernel(nc, tc, ctx_d, ques_d, ctxT_d, quesT_d, f32p_d, bf16p_d,
                  wb_d, ws_d, we_d, out_d):
    import contextlib
    est = contextlib.ExitStack()
    with est:
        const = est.enter_context(tc.tile_pool(name="const", bufs=1))
        attn = est.enter_context(tc.tile_pool(name="attn", bufs=1))
        wres = est.enter_context(tc.tile_pool(name="wres", bufs=1))
        tring = est.enter_context(tc.tile_pool(name="tring", bufs=9))
        acts = est.enter_context(tc.tile_pool(name="acts", bufs=1))
        oplane = est.enter_context(tc.tile_pool(name="oplane", bufs=3))
        cols = est.enter_context(tc.tile_pool(name="cols", bufs=2))
        pmm = est.enter_context(tc.tile_pool(name="pmm", bufs=3, space="PSUM"))
        pffw = est.enter_context(tc.tile_pool(name="pffw", bufs=3, space="PSUM"))
        ptiny = est.enter_context(tc.tile_pool(name="ptiny", bufs=2, space="PSUM"))
        pat = ptiny

        def mm(out, lhsT, rhs, start, stop):
            nc.tensor.matmul(out, lhsT, rhs, start=start, stop=stop)

        # HAM warm-up: ~3us of dependency-free PE activity (plain fp32, fed by
        # a memset tile) so the clock gate opens before the first dense phase.
        ones2_f = const.tile([128, 2], F32, tag="ones2_f")
        nc.vector.memset(ones2_f[:], 1.0)
        p_warm = pmm.tile([128, 512], F32, tag="pmm")
        for wi in range(60):
            nc.tensor.matmul(p_warm[0:2, 0:2], ones2_f[:], ones2_f[:],
                             start=(wi == 0), stop=(wi == 59))

        # ---- DMA plan: each hw queue carries one batch's critical inputs
        # FIRST (plus one const pack), then its share of the bulk weights.
        # Criticals on both queues drain in parallel at full HBM rate before
        # any bulk weight competes for bandwidth.
        f32p = const.tile([128, NF32], F32, tag="f32p")
        nc.scalar.dma_start(out=f32p[:], in_=f32p_d[:])
        bf16p = const.tile([128, NBF16], F16, tag="bf16p")
        nc.sync.dma_start(out=bf16p[:], in_=bf16p_d[:])

        colw = bf16p[:, OFF_COLW:OFF_COLW + 8]
        ident = bf16p[:, OFF_IDENT:OFF_IDENT + 128]
        ones_row = bf16p[0:1, OFF_ONES:OFF_ONES + 512]
        ones2 = bf16p[:, OFF_ONES:OFF_ONES + 2]
        wrow4 = bf16p[0:1, OFF_WROW4:OFF_WROW4 + 4]

        quesT_sb, ctxT_sb, ques_sb, ctx_sb = {}, {}, {}, {}
        for b, eng in ((0, nc.scalar), (1, nc.sync)):
            t_ = attn.tile([128, NHC, LQ], F16, tag=f"quesT_{b}", name=f"quesT_{b}")
            eng.dma_start(out=t_[:], in_=quesT_d[b])
            quesT_sb[b] = [t_[:, hc, :] for hc in range(NHC)]
            t_ = attn.tile([128, NHC, LC], F16, tag=f"ctxT_{b}", name=f"ctxT_{b}")
            eng.dma_start(out=t_[:], in_=ctxT_d[b])
            ctxT_sb[b] = [t_[:, hc, :] for hc in range(NHC)]
            q_ = attn.tile([LQ, H], F16, tag=f"ques_{b}", name=f"ques_{b}")
            eng.dma_start(out=q_[:], in_=ques_d[b, :, :])
            ques_sb[b] = q_
            t_ = attn.tile([128, NIC, H], F16, tag=f"ctx_{b}", name=f"ctx_{b}")
            eng.dma_start(out=t_[:], in_=ctx_d[b])
            ctx_sb[b] = [t_[:, ic, :] for ic in range(NIC)]

        # bulk weights, after the criticals, ALL on the sync queue ordered by
        # first use (w1s ~20us ... Wb ~80us).  The scalar (Activation) hw
        # queue only gets ~90GB/s when sync's queue is active, so bulk on it
        # arrives late; sync alone moves 10.3MB well before each deadline.
        wtiles = {}
        for lname, (w1_d, w2_d) in (("s", ws_d), ("e", we_d)):
            w1t = wres.tile([128, NDC, DFF], F16, tag=f"w1{lname}", name=f"w1{lname}")
            nc.sync.dma_start(out=w1t[:], in_=w1_d[:])
            w2t = wres.tile([128, NFC, D], F16, tag=f"w2{lname}", name=f"w2{lname}")
            nc.sync.dma_start(out=w2t[:], in_=w2_d[:])
            wtiles[lname] = (w1t, w2t)
        wbt_tile = wres.tile([128, NDC, C, D + 1], F16, tag="wb")
        nc.sync.dma_start(out=wbt_tile[:], in_=wb_d[:])

        def attention_pair():
            """Both batches' attention, instruction-interleaved stage by stage.
            Returns {b: xT chunk list} (8 tiles [128, LC] bf16 each)."""
            BS = (0, 1)
            quesT, ctxT = quesT_sb, ctxT_sb

            # (ctx*w3)^T
            ctxw3T = {b: [] for b in BS}
            for b in BS:
                for hc in range(NHC):
                    t_ = attn.tile([128, LC], F16, tag=f"ctxw3T{hc}_{b}")
                    nc.vector.tensor_scalar_mul(
                        t_[:], ctxT[b][hc],
                        f32p[:, COL_W3 + hc:COL_W3 + hc + 1])
                    ctxw3T[b].append(t_)

            # Stacked rank-2 tiles, built wholly in PSUM via zero-padded
            # weight columns + a rank-1 ones term: q2ones = [ones; ques@w2],
            # c1ones = [ctx@w1; ones].  Both broadcast terms of sim are then
            # ONE matmul: c1ones[:,isl].T @ q2ones = c1[i]*1 + 1*q2[j].
            o10 = bf16p[0:1, OFF_Z:OFF_Z + 2]
            o01 = bf16p[0:1, OFF_Z + 1:OFF_Z + 3]
            q2ones, c1ones = {}, {}
            for b in BS:
                p_q2r = pat.tile([2, LQ], F32, tag="pt", name=f"pq2r_{b}")
                for hc in range(NHC):
                    mm(p_q2r[:], colw[:, 4 + 2 * hc:6 + 2 * hc], quesT[b][hc],
                       start=(hc == 0), stop=False)
                mm(p_q2r[:], o10, ones_row[:, 0:LQ], start=False, stop=True)
                q2ones[b] = cols.tile([2, LQ], F16, tag="q2row", name=f"q2row_{b}")
                nc.scalar.activation(q2ones[b][:], p_q2r[:], AF.Copy)
            for b in BS:
                p_c1r = pffw.tile([2, LC], F32, tag="pf", name=f"pc1r_{b}")
                for hc in range(NHC):
                    mm(p_c1r[:], colw[:, 2 * hc:2 * hc + 2], ctxT[b][hc],
                       start=(hc == 0), stop=False)
                mm(p_c1r[:], o01, ones_row[:, 0:LC], start=False, stop=True)
                c1ones[b] = cols.tile([2, LC], F16, tag="c1row", name=f"c1row_{b}")
                nc.scalar.activation(c1ones[b][:], p_c1r[:], AF.Copy)

            ucols = {b: attn.tile([128, NIC + 2], F16, tag=f"ucols_{b}",
                                  name=f"ucols_{b}") for b in BS}
            a_n = {b: [] for b in BS}
            for ic in range(NIC):
                icsl = slice(ic * 128, (ic + 1) * 128)
                for b in BS:
                    p_sim = pmm.tile([128, LQ], F32, tag="pmm", name=f"psim_{b}{ic}")
                    for hc in range(NHC):
                        mm(p_sim[:], ctxw3T[b][hc][:, icsl], quesT[b][hc],
                           start=(hc == 0), stop=False)
                    mm(p_sim[:], c1ones[b][:, icsl], q2ones[b][:],
                       start=False, stop=True)

                    a_un = attn.tile([128, LQ], F32, tag=f"aun{ic}_{b}")
                    nc.scalar.activation(a_un[:], p_sim[:], AF.Exp)
                    ssum = cols.tile([128, 1], F32, tag="ssum", name=f"ssum_{b}{ic}")
                    nc.vector.reduce_sum(out=ssum[:], in_=a_un[:], axis=AX.X)
                    srec = cols.tile([128, 1], F32, tag="srec", name=f"srec_{b}{ic}")
                    nc.vector.reciprocal(srec[:], ssum[:])
                    nc.vector.reduce_max(out=ucols[b][:, ic:ic + 1], in_=a_un[:], axis=AX.X)
                    t_ = attn.tile([128, LQ], F16, tag=f"an{ic}_{b}")
                    nc.vector.tensor_scalar_mul(t_[:], a_un[:], srec[:])
                    a_n[b].append(t_)

            # a^T [j-part, i-free]
            aT = {b: attn.tile([LQ, LC], F16, tag=f"aT_{b}", name=f"aT_{b}")
                  for b in BS}
            for b in BS:
                for ic in range(NIC):
                    p = pffw.tile([LQ, 128], F16, tag="pf", name=f"paT_{b}{ic}")
                    nc.tensor.transpose(p[:], a_n[b][ic][:], ident)
                    nc.scalar.activation(aT[b][:, ic * 128:(ic + 1) * 128], p[:], AF.Copy)

            # softmax-over-i weights: denominator + broadcast of 1/den
            invb = {}
            for b in BS:
                ucol1 = cols.tile([128, 1], F16, tag="ucol1", name=f"ucol1_{b}")
                with nc.allow_low_precision(reason="4-term bf16 softmax-denominator partial sum"):
                    nc.vector.reduce_sum(out=ucol1[:], in_=ucols[b][:, 0:NIC], axis=AX.X)
                p_den = pat.tile([1, 2], F32, tag="pt", name=f"pden_{b}")
                mm(p_den[:], ucol1[:], ones2, start=True, stop=True)
                inv2f = cols.tile([1, 2], F32, tag="inv2f", name=f"inv2f_{b}")
                nc.vector.reciprocal(inv2f[:], p_den[:])
                inv2 = cols.tile([1, 2], F16, tag="inv2", name=f"inv2_{b}")
                nc.scalar.activation(inv2[:], inv2f[:], AF.Copy)
                p_bc = pat.tile([128, 2], F32, tag="pt", name=f"pbc_{b}")
                mm(p_bc[:], ones_row[:, 0:128], inv2[:], start=True, stop=True)
                invb[b] = cols.tile([128, 1], F32, tag="invb", name=f"invb_{b}")
                nc.scalar.activation(invb[b][:], p_bc[:, 0:1], AF.Copy)

            q2cc = {b: [] for b in BS}
            for b in BS:
                for hs in range(NHC):
                    p_q2c = pat.tile([128, 2], F32, tag="pt", name=f"pq2c_{b}{hs}")
                    for ic in range(NIC):
                        mm(p_q2c[:], ctx_sb[b][ic][:, hs * 128:(hs + 1) * 128],
                           ucols[b][:, ic:ic + 2], start=(ic == 0), stop=(ic == NIC - 1))
                    t_ = cols.tile([128, 1], F32, tag=f"q2cc{hs}", name=f"q2cc_{b}{hs}")
                    nc.vector.tensor_mul(t_[:], p_q2c[:, 0:1], invb[b][:])
                    q2cc[b].append(t_)

            # x^T chunks: 0-1 ctx^T, 2-3 c2q^T, 4-5 (ctx*c2q)^T, 6-7 (ctx*q2c)^T
            xT = {}
            for b in BS:
                xT[b] = [ctxT[b][0], ctxT[b][1]]
                for hs in range(NHC):
                    p_c2q = pffw.tile([128, LC], F32, tag="pf", name=f"pc2q_{b}{hs}")
                    mm(p_c2q[:], ques_sb[b][:, hs * 128:(hs + 1) * 128], aT[b][:],
                       start=True, stop=True)
                    t_ = acts.tile([128, LC], F16, tag=f"xT{2 + hs}_{b}")
                    nc.scalar.activation(t_[:], p_c2q[:], AF.Copy)
                    xT[b].append(t_)
                for hc in range(NHC):
                    t_ = acts.tile([128, LC], F16, tag=f"xT{4 + hc}_{b}")
                    nc.vector.tensor_mul(t_[:], ctxT[b][hc], xT[b][2 + hc][:])
                    xT[b].append(t_)
                for hc in range(NHC):
                    t_ = acts.tile([128, LC], F16, tag=f"xT{6 + hc}_{b}")
                    nc.vector.tensor_scalar_mul(t_[:], ctxT[b][hc], q2cc[b][hc][:])
                    xT[b].append(t_)
            return xT

        def ffw(b, xT):
            sT, eT = [], []
            for lname, colb1, colb2, dst in (
                ("s", COL_B1S, COL_B2S, sT),
                ("e", COL_B1E, COL_B2E, eT),
            ):
                w1t, w2t = wtiles[lname]
                h1 = []
                dc_order = [0, 1, 6, 7, 2, 3, 4, 5]
                for fc in range(NFC):
                    p = pffw.tile([128, LC], F32, tag="pf", name=f"ph1{lname}_{b}{fc}")
                    for k, dc in enumerate(dc_order):
                        mm(p[:], w1t[:, dc, fc * 128:(fc + 1) * 128], xT[dc],
                           start=(k == 0), stop=(k == NDC - 1))
                    t_ = acts.tile([128, LC], F16, tag=f"h1{fc}",
                                   name=f"h1{lname}{fc}_{b}")
                    nc.vector.tensor_scalar(
                        out=t_[:], in0=p[:],
                        scalar1=f32p[:, colb1 + fc:colb1 + fc + 1],
                        scalar2=0.0, op0=ALU.add, op1=ALU.max)
                    h1.append(t_)
                for dc in range(NDC):
                    p = pffw.tile([128, LC], F32, tag="pf", name=f"po{lname}_{b}{dc}")
                    for fc in range(NFC):
                        mm(p[:], w2t[:, fc, dc * 128:(dc + 1) * 128], h1[fc][:],
                           start=(fc == 0), stop=(fc == NFC - 1))
                    t_ = acts.tile([128, LC], F16, tag=f"{lname}T{dc}", bufs=2,
                                   name=f"{lname}T{dc}_{b}")
                    nc.scalar.activation(
                        t_[:], p[:], AF.Identity,
                        bias=f32p[:, colb2 + dc:colb2 + dc + 1],
                        scale=1.0)
                    dst.append(t_)
            return sT, eT

        def biaffine(b, sT, eT):
            # t1 rows for all three labels in one group:
            # t1[c, x] = sum_i start^T[i, x] * Wb[i, c, D]  + Wb[D, c, D]
            p_t14 = pffw.tile([4, LC], F32, tag="pf", name=f"pt14_{b}")
            for ic in range(NDC):
                mm(p_t14[:], bf16p[:, OFF_UPACK + ic * 4:OFF_UPACK + ic * 4 + 4],
                   sT[ic][:], start=(ic == 0), stop=False)
            mm(p_t14[:], wrow4, ones_row, start=False, stop=True)
            t14 = cols.tile([4, LC], F16, tag="t14", name=f"t14_{b}")
            nc.scalar.activation(t14[:], p_t14[:], AF.Copy)
            t1cols = []
            for xc in range(NIC):
                p = ptiny.tile([128, 4], F16, tag="pt", name=f"pt1c_{b}{xc}")
                nc.tensor.transpose(p[:], t14[:, xc * 128:(xc + 1) * 128],
                                    ident[0:4, 0:4])
                tsb = cols.tile([128, 4], F32, tag=f"t1c{xc}", name=f"t1c{xc}_{b}")
                nc.vector.tensor_copy(tsb[:], p[:])
                t1cols.append(tsb)

            for c in range(C):
                # t_c^T[j, x] = sum_i Wb[i,c,j] * start^T[i, x]  (+ v_c[j])
                tt = []
                for jc in range(NJC):
                    p = pmm.tile([128, LC], F32, tag="pmm", name=f"pt_{b}{c}{jc}")
                    for ic in range(NDC):
                        mm(p[:], wbt_tile[:, ic, c, jc * 128:(jc + 1) * 128], sT[ic][:],
                           start=(ic == 0), stop=(ic == NDC - 1))
                    t_ = tring.tile([128, LC], F16, tag="t", name=f"t_{b}{c}{jc}")
                    nc.vector.tensor_scalar_add(
                        t_[:], p[:],
                        f32p[:, COL_VC + c * NJC + jc:COL_VC + c * NJC + jc + 1])
                    tt.append(t_)

                # score_c[x, y] = sum_j t_c^T[j, x] * end^T[j, y] + t1_c[x],
                # accumulated into a [128, 4, LC] bf16 plane group, DMA'd out
                # as one [LC, LC] label plane on the scalar hardware queue.
                planes = oplane.tile([128, NIC, LC], F16, tag="opl",
                                     name=f"opl_{b}{c}")
                last = (b == NB - 1 and c == C - 1)
                for xc in range(NIC):
                    p = pmm.tile([128, LC], F32, tag="pmm", name=f"ps_{b}{c}{xc}")
                    for jc in range(NJC):
                        mm(p[:], tt[jc][:, xc * 128:(xc + 1) * 128], eT[jc][:],
                           start=(jc == 0), stop=(jc == NJC - 1))
                    if last and xc % 2 == 1:
                        nc.vector.tensor_scalar_add(planes[:, xc, :], p[:],
                                                    t1cols[xc][:, c:c + 1])
                    else:
                        nc.scalar.activation(planes[:, xc, :], p[:], AF.Identity,
                                             bias=t1cols[xc][:, c:c + 1], scale=1.0)
                    if last:
                        nc.sync.dma_start(out=out_d[b, c, :, xc, :],
                                          in_=planes[:, xc, :])
                if not last:
                    nc.sync.dma_start(out=out_d[b, c], in_=planes[:])

        # ---- phase-interleaved schedule ----
        # A0+A1 interleaved, then both FFWs, then both biaffines (sT/eT are
        # double-buffered), so the PE stream never stalls on front-end work
        # mid-kernel.
        xT = attention_pair()
        se0 = ffw(0, xT[0])
        se1 = ffw(1, xT[1])
        biaffine(0, *se0)
        biaffine(1, *se1)


_PROGRAM_CACHE = {}


def _get_program():
    if "nc" not in _PROGRAM_CACHE:
        _PROGRAM_CACHE["nc"] = _build_program()
    return _PROGRAM_CACHE["nc"]


def _pack_host_inputs(w_sim, W1s, b1s, W2s, b2s, W1e, b1e, W2e, b2e, Wb):
    """Build the shared (replicated) input arrays from the raw weights."""
    import ml_dtypes
    f32, f16 = np.float32, ml_dtypes.bfloat16
    w1, w2, w3 = [np.asarray(w_sim[k * H:(k + 1) * H], f32) for k in range(3)]

    f32p = np.zeros((128, NF32), f32)
    for hc in range(NHC):
        f32p[:, COL_W3 + hc] = w3[hc * 128:(hc + 1) * 128]
    for fc in range(NFC):
        f32p[:, COL_B1S + fc] = b1s[fc * 128:(fc + 1) * 128]
        f32p[:, COL_B1E + fc] = b1e[fc * 128:(fc + 1) * 128]
    for dc in range(NDC):
        f32p[:, COL_B2S + dc] = b2s[dc * 128:(dc + 1) * 128]
        f32p[:, COL_B2E + dc] = b2e[dc * 128:(dc + 1) * 128]
    for c in range(C):
        for jc in range(NJC):
            f32p[:, COL_VC + c * NJC + jc] = Wb[D, c, jc * 128:(jc + 1) * 128]

    bf16p = np.zeros((128, NBF16), f32)
    bf16p[:, OFF_ONES:OFF_ONES + 512] = 1.0
    for hc in range(NHC):
        bf16p[:, OFF_COLW + 2 * hc] = w1[hc * 128:(hc + 1) * 128]
        bf16p[:, OFF_COLW + 5 + 2 * hc] = w2[hc * 128:(hc + 1) * 128]
    bf16p[:, OFF_Z] = 1.0
    bf16p[:, OFF_Z + 2] = 1.0
    bf16p[:, OFF_IDENT:OFF_IDENT + 128] = np.eye(128, dtype=f32)
    for dc in range(NDC):
        for c in range(C):
            bf16p[:, OFF_UPACK + dc * 4 + c] = Wb[dc * 128:(dc + 1) * 128, c, D]
    bf16p[:, OFF_WROW4:OFF_WROW4 + C] = Wb[D, :, D][None, :]

    def pmaj(a, nchunk):
        # [nchunk*128, F...] -> [128, nchunk, F...] (SBUF-tile layout)
        return np.ascontiguousarray(
            a.reshape((nchunk, 128) + a.shape[1:]).swapaxes(0, 1))

    return {
        "f32pack": f32p,
        "bf16pack": bf16p.astype(f16),
        "wb": pmaj(Wb[:D].astype(f16), NDC),
        "W1s": pmaj(W1s.astype(f16), NDC),
        "W2s": pmaj(W2s.astype(f16), NFC),
        "W1e": pmaj(W1e.astype(f16), NDC),
        "W2e": pmaj(W2e.astype(f16), NFC),
    }


def kernel(ctx_emb, ques_emb, w_sim, W1s, b1s, W2s, b2s, W1e, b1e, W2e, b2e, Wb,
           _trace=False, _tmpdir=None):
    from concourse.bass_utils import run_bass_kernel_spmd
    import ml_dtypes

    # accept jax/np arrays of any layout
    (ctx_emb, ques_emb, w_sim, W1s, b1s, W2s, b2s, W1e, b1e, W2e, b2e, Wb) = (
        np.asarray(a, dtype=np.float32)
        for a in (ctx_emb, ques_emb, w_sim, W1s, b1s, W2s, b2s, W1e, b1e, W2e,
                  b2e, Wb))

    nc = _get_program()
    shared = _pack_host_inputs(w_sim, W1s, b1s, W2s, b2s, W1e, b1e, W2e, b2e, Wb)
    ctx16 = ctx_emb.astype(ml_dtypes.bfloat16)
    ques16 = np.ascontiguousarray(ques_emb.astype(ml_dtypes.bfloat16))
    # p-major repacks matching the SBUF tile layouts ([.., 128, chunk, free])
    ctxp = np.ascontiguousarray(
        ctx16.reshape(B, NIC, 128, H).swapaxes(1, 2))            # [B,128,4,H]
    ctxTp = np.ascontiguousarray(
        ctx16.transpose(0, 2, 1).reshape(B, NHC, 128, LC).swapaxes(1, 2))
    quesTp = np.ascontiguousarray(
        ques16.transpose(0, 2, 1).reshape(B, NHC, 128, LQ).swapaxes(1, 2))
    in_maps = []
    for core in range(N_CORES):
        sl = slice(core * NB, (core + 1) * NB)
        in_maps.append({"ctx": ctxp[sl], "ques": ques16[sl],
                        "ctxT": ctxTp[sl], "quesT": quesTp[sl], **shared})

    kw = {}
    if _trace:
        kw = {"trace": True, "tmpdir": _tmpdir}
    res = run_bass_kernel_spmd(nc, in_maps, list(range(N_CORES)), **kw)
    # device layout is [NB, C, 128, NIC, LC] bf16 (p-major planes);
    # upcast + unpermute to [B, LC, LC, C] on the host.
    outs = []
    for i in range(N_CORES):
        o = np.asarray(res.results[i]["out"])  # [NB, C, 128, NIC, LC]
        o = o.astype(np.float32).transpose(0, 3, 2, 4, 1)  # [NB, NIC, 128, LC, C]
        outs.append(o.reshape(NB, LC, LC, C))
    out = np.ascontiguousarray(np.concatenate(outs, axis=0))
    if _trace:
        return out, res
    return out
